# revision 3
# baseline (speedup 1.0000x reference)
"""Trainium2 Bass kernel for nn_DS2_62466004353281.

Computes out[b,h,w,i,c] = inputs[b,h,w,i] * u[i,c], with
u = beta^2 / rowsum(beta^2) computed on device from beta [64,16].

Sharding: data-parallel over batch B=8 -> one batch image per NeuronCore.
Per core: x [128,128,64] f32 (4 MiB in), out [128,128,64,16] f32 (64 MiB out).
Memory-bound on the output writes.

Layout: SBUF partition dim = H (128). x lives as [128, W*D] (free=(w,i)).
u is expanded to u_rep8 [128, WCHUNK*D*C] (replicated across partitions
and across the w's of a chunk), so each w-chunk's multiply is ONE DVE
tensor_tensor with 2 free dims:
    out[p, (wi):step16, c:step1] = x[p, (wi):step1, c:step0] * u_rep8[...]
(the walrus TT struct carries no attached sem waits, hence raw Bass with
standalone wait_ge instructions rather than Tile).

Output DMAs alternate between the two HWDGE rings (sync=SP, scalar=ACT),
4 MiB each, fully contiguous 32 KiB per-partition runs.
"""

import sys

if "/opt/trn_rl_repo" not in sys.path:
    sys.path.insert(0, "/opt/trn_rl_repo")

import numpy as np

import concourse.bass as bass
from concourse import mybir
from concourse.bass_utils import run_bass_kernel_spmd

B, H, W, D, C = 8, 128, 128, 64, 16
N_CORES = 8
WCHUNK = 8          # w's per output tile -> tile [128, 8192] = 32 KiB/partition
NBUF = 4            # output tiles in flight (2 per HWDGE ring)
F32 = mybir.dt.float32


def build_bass(w_total=W, repeat=1):
    """Per-core Bass module. w_total/repeat are knobs for small-scale
    simulation and on-HW timing via kernel repetition (the kernel is
    idempotent, so repeats write identical output)."""
    n_chunks = w_total // WCHUNK
    assert n_chunks >= 1 and w_total % WCHUNK == 0
    # x loaded in 2 halves so compute can start after the first half
    xh = max(n_chunks // 2, 1)  # chunks covered by first x-DMA

    nc = bass.Bass()
    x = nc.declare_dram_parameter("x", [H, w_total * D], F32, isOutput=False)
    beta = nc.declare_dram_parameter("beta", [D, C], F32, isOutput=False)
    out = nc.declare_dram_parameter("out", [H, w_total * D * C], F32, isOutput=True)
    u_dram = nc.dram_tensor("u_scratch", [D * C], F32)

    csz = WCHUNK * D * C  # elems per out chunk per partition row group
    with (
        nc.sbuf_tensor([H, w_total * D], F32) as xt,
        nc.sbuf_tensor([D, C], F32) as beta_t,
        nc.sbuf_tensor([D, C], F32) as b2,
        nc.sbuf_tensor([D, 1], F32) as ssum,
        nc.sbuf_tensor([D, 1], F32) as rinv,
        nc.sbuf_tensor([D, C], F32) as u_small,
        nc.sbuf_tensor([H, D * C], F32) as u_rep,
        nc.sbuf_tensor([H, NBUF * csz], F32) as ot,
        nc.sbuf_tensor([H, csz], F32) as u_rep8,
        nc.semaphore("s_beta") as s_beta,
        nc.semaphore("s_x0") as s_x0,
        nc.semaphore("s_x1") as s_x1,
        nc.semaphore("s_ub") as s_ub,
        nc.semaphore("s_ur") as s_ur,
        nc.semaphore("v_sem") as v_sem,
        nc.semaphore("out_e") as out_e,
        nc.semaphore("out_o") as out_o,
        nc.Block() as block,
    ):
        PRO = 5  # DVE prologue ops before the first TT (4 u-chain + u_rep8)

        @block.gpsimd
        def _(gpsimd):
            gpsimd.dma_start(beta_t[:], beta[:]).then_inc(s_beta, 16)
            gpsimd.dma_start(
                xt[:, : xh * WCHUNK * D], x[:, : xh * WCHUNK * D]
            ).then_inc(s_x0, 16)
            if n_chunks > xh:
                gpsimd.dma_start(
                    xt[:, xh * WCHUNK * D :], x[:, xh * WCHUNK * D :]
                ).then_inc(s_x1, 16)
            # u bounce: SBUF [64,16] -> DRAM flat [1024]
            gpsimd.wait_ge(v_sem, 4)
            gpsimd.dma_start(u_dram[:], u_small[:]).then_inc(s_ub, 16)
            gpsimd.wait_ge(s_ub, 16)
            # partition-broadcast read back: [1024] -> [128, 1024]
            gpsimd.dma_start(
                u_rep[:], u_dram[:].unsqueeze(0).broadcast_to([H, D * C])
            ).then_inc(s_ur, 16)

        @block.vector
        def _(vector):
            vector.wait_ge(s_beta, 16)
            vector.tensor_mul(b2[:], beta_t[:], beta_t[:]).then_inc(v_sem)
            vector.wait_ge(v_sem, 1)  # DVE pipeline RAW drain
            vector.reduce_sum(ssum[:], b2[:], axis=mybir.AxisListType.X).then_inc(
                v_sem
            )
            vector.wait_ge(v_sem, 2)
            vector.reciprocal(rinv[:], ssum[:]).then_inc(v_sem)
            vector.wait_ge(v_sem, 3)
            vector.tensor_scalar(
                u_small[:], b2[:], rinv[:], None, op0=mybir.AluOpType.mult
            ).then_inc(v_sem)
            vector.wait_ge(s_ur, 16)
            # replicate u across the WCHUNK w's of a chunk: [128, 8*1024]
            vector.tensor_copy(
                u_rep8[:].rearrange("p (w e) -> p w e", w=WCHUNK),
                u_rep[:].unsqueeze(1).broadcast_to([H, WCHUNK, D * C]),
            ).then_inc(v_sem)
            vector.wait_ge(v_sem, PRO)  # u_rep8 drained before first TT reads it
            for r in range(repeat):
                for k in range(n_chunks):
                    if r == 0 and k == 0:
                        vector.wait_ge(s_x0, 16)
                    if r == 0 and k == xh and n_chunks > xh:
                        vector.wait_ge(s_x1, 16)
                    kk = r * n_chunks + k
                    if kk >= NBUF:
                        # slot reuse: wait for the DMA that drained this slot
                        # (chunk kk-NBUF, same ring since NBUF is even)
                        prev = kk - NBUF
                        sem = out_e if prev % 2 == 0 else out_o
                        vector.wait_ge(sem, 16 * (prev // 2 + 1))
                    slot = kk % NBUF
                    in0 = (
                        xt[:, k * WCHUNK * D : (k + 1) * WCHUNK * D]
                        .unsqueeze(2)
                        .broadcast_to([H, WCHUNK * D, C])
                    )
                    in1 = u_rep8[:].rearrange("p (wi c) -> p wi c", c=C)
                    o2 = ot[:, slot * csz : (slot + 1) * csz].rearrange(
                        "p (wi c) -> p wi c", c=C
                    )
                    vector.tensor_mul(o2, in0, in1).then_inc(v_sem)

        @block.sync
        def _(sync):
            for r in range(repeat):
                for k in range(0, n_chunks, 2):
                    kk = r * n_chunks + k
                    sync.wait_ge(v_sem, PRO + kk + 1)
                    slot = kk % NBUF
                    sync.dma_start(
                        out[:, k * csz : (k + 1) * csz],
                        ot[:, slot * csz : (slot + 1) * csz],
                    ).then_inc(out_e, 16)
            n_e = sum(
                1 for r in range(repeat) for k in range(0, n_chunks, 2)
            )
            sync.wait_ge(out_e, 16 * n_e)

        @block.scalar
        def _(scalar):
            for r in range(repeat):
                for k in range(1, n_chunks, 2):
                    kk = r * n_chunks + k
                    scalar.wait_ge(v_sem, PRO + kk + 1)
                    slot = kk % NBUF
                    scalar.dma_start(
                        out[:, k * csz : (k + 1) * csz],
                        ot[:, slot * csz : (slot + 1) * csz],
                    ).then_inc(out_o, 16)
            n_o = sum(
                1 for r in range(repeat) for k in range(1, n_chunks, 2)
            )
            if n_o:
                scalar.wait_ge(out_o, 16 * n_o)

    return nc


_NC_CACHE = {}


def get_bass(w_total=W, repeat=1):
    key = (w_total, repeat)
    if key not in _NC_CACHE:
        _NC_CACHE[key] = build_bass(w_total=w_total, repeat=repeat)
    return _NC_CACHE[key]


def kernel(inputs, beta):
    inputs = np.ascontiguousarray(inputs, dtype=np.float32)
    beta = np.ascontiguousarray(beta, dtype=np.float32)
    assert inputs.shape == (B, H, W, D), inputs.shape
    assert beta.shape == (D, C), beta.shape

    nc = get_bass()
    in_maps = [
        {"x": inputs[b].reshape(H, W * D), "beta": beta} for b in range(B)
    ]
    res = run_bass_kernel_spmd(nc, in_maps, core_ids=list(range(N_CORES)))
    return np.stack(
        [res.results[b]["out"].reshape(H, W, D, C) for b in range(B)], axis=0
    )


# revision 6
# speedup vs baseline: 2.9421x; 2.9421x over previous
"""Trainium2 Bass kernel for nn_DS2_62466004353281.

Computes out[b,h,w,i,c] = inputs[b,h,w,i] * u[i,c], with
u = beta^2 / rowsum(beta^2) computed on device from beta [64,16].

Sharding: data-parallel over batch B=8 -> one batch image per NeuronCore.
Per core: x [128,128,64] f32 (4 MiB in), out [128,128,64,16] f32 (64 MiB out).
Memory-bound on the output writes.

Layout: SBUF partition dim = H (128). x lives as [128, W*D] (free=(w,i)).
u is expanded to u_rep8 [128, WCHUNK*D*C] (replicated across partitions
and across the w's of a chunk), so each w-chunk's multiply is ONE DVE
tensor_tensor with 2 free dims:
    out[p, (wi):step16, c:step1] = x[p, (wi):step1, c:step0] * u_rep8[...]
(the walrus TT struct carries no attached sem waits, hence raw Bass with
standalone wait_ge instructions rather than Tile).

Output DMAs alternate between the two HWDGE rings (sync=SP, scalar=ACT),
4 MiB each, fully contiguous 32 KiB per-partition runs.
"""

import sys

if "/opt/trn_rl_repo" not in sys.path:
    sys.path.insert(0, "/opt/trn_rl_repo")

import numpy as np

import concourse.bass as bass
from concourse import mybir
from concourse.bass_utils import run_bass_kernel_spmd

B, H, W, D, C = 8, 128, 128, 64, 16
N_CORES = 8
WCHUNK = 8          # w's per output tile -> tile [128, 8192] = 32 KiB/partition
NBUF = 4            # output tiles in flight (2 per HWDGE ring)
F32 = mybir.dt.float32


def build_bass(w_total=W, repeat=1):
    """Per-core Bass module. w_total/repeat are knobs for small-scale
    simulation and on-HW timing via kernel repetition (the kernel is
    idempotent, so repeats write identical output)."""
    n_chunks = w_total // WCHUNK
    assert n_chunks >= 1 and w_total % WCHUNK == 0
    # x loaded in 2 halves so compute can start after the first half
    xh = max(n_chunks // 2, 1)  # chunks covered by first x-DMA

    nc = bass.Bass()
    x = nc.declare_dram_parameter("x", [H, w_total * D], F32, isOutput=False)
    beta = nc.declare_dram_parameter("beta", [D, C], F32, isOutput=False)
    out = nc.declare_dram_parameter("out", [H, w_total * D * C], F32, isOutput=True)
    u_dram = nc.dram_tensor("u_scratch", [D * C], F32)

    csz = WCHUNK * D * C  # elems per out chunk per partition row group
    with (
        nc.sbuf_tensor([H, w_total * D], F32) as xt,
        nc.sbuf_tensor([D, C], F32) as beta_t,
        nc.sbuf_tensor([D, C], F32) as b2,
        nc.sbuf_tensor([D, 1], F32) as ssum,
        nc.sbuf_tensor([D, 1], F32) as rinv,
        nc.sbuf_tensor([D, C], F32) as u_small,
        nc.sbuf_tensor([H, D * C], F32) as u_rep,
        nc.sbuf_tensor([H, NBUF * csz], F32) as ot,
        nc.sbuf_tensor([H, csz], F32) as u_rep8,
        nc.semaphore("s_beta") as s_beta,
        nc.semaphore("s_x0") as s_x0,
        nc.semaphore("s_x1") as s_x1,
        nc.semaphore("s_ub") as s_ub,
        nc.semaphore("s_ur") as s_ur,
        nc.semaphore("v_sem") as v_sem,
        nc.semaphore("out_s0") as out_s0,
        nc.semaphore("out_s1") as out_s1,
        nc.semaphore("out_s2") as out_s2,
        nc.semaphore("out_s3") as out_s3,
        nc.Block() as block,
    ):
        out_sems = [out_s0, out_s1, out_s2, out_s3]
        PRO = 5  # DVE prologue ops before the first TT (4 u-chain + u_rep8)

        @block.gpsimd
        def _(gpsimd):
            gpsimd.dma_start(beta_t[:], beta[:]).then_inc(s_beta, 16)
            gpsimd.dma_start(
                xt[:, : xh * WCHUNK * D], x[:, : xh * WCHUNK * D]
            ).then_inc(s_x0, 16)
            if n_chunks > xh:
                gpsimd.dma_start(
                    xt[:, xh * WCHUNK * D :], x[:, xh * WCHUNK * D :]
                ).then_inc(s_x1, 16)
            # u bounce: SBUF [64,16] -> DRAM flat [1024]
            gpsimd.wait_ge(v_sem, 4)
            gpsimd.dma_start(u_dram[:], u_small[:]).then_inc(s_ub, 16)
            gpsimd.wait_ge(s_ub, 16)
            # partition-broadcast read back: [1024] -> [128, 1024]
            gpsimd.dma_start(
                u_rep[:], u_dram[:].unsqueeze(0).broadcast_to([H, D * C])
            ).then_inc(s_ur, 16)

        @block.vector
        def _(vector):
            vector.wait_ge(s_beta, 16)
            vector.tensor_mul(b2[:], beta_t[:], beta_t[:]).then_inc(v_sem)
            vector.wait_ge(v_sem, 1)  # DVE pipeline RAW drain
            vector.reduce_sum(ssum[:], b2[:], axis=mybir.AxisListType.X).then_inc(
                v_sem
            )
            vector.wait_ge(v_sem, 2)
            vector.reciprocal(rinv[:], ssum[:]).then_inc(v_sem)
            vector.wait_ge(v_sem, 3)
            vector.tensor_scalar(
                u_small[:], b2[:], rinv[:], None, op0=mybir.AluOpType.mult
            ).then_inc(v_sem)
            vector.wait_ge(s_ur, 16)
            # replicate u across the WCHUNK w's of a chunk: [128, 8*1024]
            vector.tensor_copy(
                u_rep8[:].rearrange("p (w e) -> p w e", w=WCHUNK),
                u_rep[:].unsqueeze(1).broadcast_to([H, WCHUNK, D * C]),
            ).then_inc(v_sem)
            vector.wait_ge(v_sem, PRO)  # u_rep8 drained before first TT reads it
            for r in range(repeat):
                for k in range(n_chunks):
                    if r == 0 and k == 0:
                        vector.wait_ge(s_x0, 16)
                    if r == 0 and k == xh and n_chunks > xh:
                        vector.wait_ge(s_x1, 16)
                    kk = r * n_chunks + k
                    if kk >= NBUF:
                        # slot reuse: wait until this slot's previous DMA
                        # (chunk kk-NBUF) fully drained it
                        vector.wait_ge(out_sems[kk % NBUF], 16 * (kk // NBUF))
                    slot = kk % NBUF
                    in0 = (
                        xt[:, k * WCHUNK * D : (k + 1) * WCHUNK * D]
                        .unsqueeze(2)
                        .broadcast_to([H, WCHUNK * D, C])
                    )
                    in1 = u_rep8[:].rearrange("p (wi c) -> p wi c", c=C)
                    o2 = ot[:, slot * csz : (slot + 1) * csz].rearrange(
                        "p (wi c) -> p wi c", c=C
                    )
                    vector.tensor_mul(o2, in0, in1).then_inc(v_sem)

        def store_stream(eng, parity):
            cnt = [0, 0, 0, 0]
            for r in range(repeat):
                for k in range(parity, n_chunks, 2):
                    kk = r * n_chunks + k
                    eng.wait_ge(v_sem, PRO + kk + 1)
                    slot = kk % NBUF
                    eng.dma_start(
                        out[:, k * csz : (k + 1) * csz],
                        ot[:, slot * csz : (slot + 1) * csz],
                    ).then_inc(out_sems[slot], 16)
                    cnt[slot] += 1
            for s in range(NBUF):
                if cnt[s]:
                    eng.wait_ge(out_sems[s], 16 * cnt[s])

        @block.sync
        def _(sync):
            store_stream(sync, 0)

        @block.scalar
        def _(scalar):
            if n_chunks > 1:
                store_stream(scalar, 1)

    return nc


_NC_CACHE = {}


def get_bass(w_total=W, repeat=1):
    key = (w_total, repeat)
    if key not in _NC_CACHE:
        _NC_CACHE[key] = build_bass(w_total=w_total, repeat=repeat)
    return _NC_CACHE[key]


def kernel(inputs, beta):
    inputs = np.ascontiguousarray(inputs, dtype=np.float32)
    beta = np.ascontiguousarray(beta, dtype=np.float32)
    assert inputs.shape == (B, H, W, D), inputs.shape
    assert beta.shape == (D, C), beta.shape

    nc = get_bass()
    in_maps = [
        {"x": inputs[b].reshape(H, W * D), "beta": beta} for b in range(B)
    ]
    res = run_bass_kernel_spmd(nc, in_maps, core_ids=list(range(N_CORES)))
    return np.stack(
        [res.results[b]["out"].reshape(H, W, D, C) for b in range(B)], axis=0
    )


# revision 7
# speedup vs baseline: 5.4776x; 1.8618x over previous
"""Trainium2 Bass kernel for nn_DS2_62466004353281.

Computes out[b,h,w,i,c] = inputs[b,h,w,i] * u[i,c], with
u = beta^2 / rowsum(beta^2) computed on device from beta [64,16].

Sharding: data-parallel over batch B=8 -> one batch image per NeuronCore.
Per core: x [128,128,64] f32 (4 MiB in), out [128,128,64,16] f32 (64 MiB out).
Memory-bound on the output writes.

Layout: SBUF partition dim = H (128). x lives as [128, W*D] (free=(w,i)).
u is expanded to u_rep8 [128, WCHUNK*D*C] (replicated across partitions
and across the w's of a chunk), so each w-chunk's multiply is ONE DVE
tensor_tensor with 2 free dims:
    out[p, (wi):step16, c:step1] = x[p, (wi):step1, c:step0] * u_rep8[...]
(the walrus TT struct carries no attached sem waits, hence raw Bass with
standalone wait_ge instructions rather than Tile).

Output DMAs alternate between the two HWDGE rings (sync=SP, scalar=ACT),
4 MiB each, fully contiguous 32 KiB per-partition runs.
"""

import sys

if "/opt/trn_rl_repo" not in sys.path:
    sys.path.insert(0, "/opt/trn_rl_repo")

import numpy as np

import concourse.bass as bass
from concourse import mybir
from concourse.bass_utils import run_bass_kernel_spmd

B, H, W, D, C = 8, 128, 128, 64, 16
N_CORES = 8
WCHUNK = 8          # w's per output tile -> tile [128, 8192] = 32 KiB/partition
NBUF = 4            # output tiles in flight (2 per HWDGE ring)
F32 = mybir.dt.float32


def build_bass(w_total=W, repeat=1):
    """Per-core Bass module. w_total/repeat are knobs for small-scale
    simulation and on-HW timing via kernel repetition (the kernel is
    idempotent, so repeats write identical output)."""
    n_chunks = w_total // WCHUNK
    assert n_chunks >= 1 and w_total % WCHUNK == 0
    # x loaded in 2 halves so compute can start after the first half
    xh = max(n_chunks // 2, 1)  # chunks covered by first x-DMA

    nc = bass.Bass()
    x = nc.declare_dram_parameter("x", [H, w_total * D], F32, isOutput=False)
    beta = nc.declare_dram_parameter("beta", [D, C], F32, isOutput=False)
    out = nc.declare_dram_parameter("out", [H, w_total * D * C], F32, isOutput=True)
    u_dram = nc.dram_tensor("u_scratch", [D * C], F32)

    csz = WCHUNK * D * C  # elems per out chunk per partition row group
    with (
        nc.sbuf_tensor([H, w_total * D], F32) as xt,
        nc.sbuf_tensor([D, C], F32) as beta_t,
        nc.sbuf_tensor([D, C], F32) as b2,
        nc.sbuf_tensor([D, 1], F32) as ssum,
        nc.sbuf_tensor([D, 1], F32) as rinv,
        nc.sbuf_tensor([D, C], F32) as u_small,
        nc.sbuf_tensor([H, D * C], F32) as u_rep,
        nc.sbuf_tensor([H, NBUF * csz], F32) as ot,
        nc.sbuf_tensor([H, csz], F32) as u_rep8,
        nc.semaphore("s_beta") as s_beta,
        nc.semaphore("s_x0") as s_x0,
        nc.semaphore("s_x1") as s_x1,
        nc.semaphore("s_ub") as s_ub,
        nc.semaphore("s_ur") as s_ur,
        nc.semaphore("v_sem") as v_sem,
        nc.semaphore("out_s0") as out_s0,
        nc.semaphore("out_s1") as out_s1,
        nc.semaphore("out_s2") as out_s2,
        nc.semaphore("out_s3") as out_s3,
        nc.Block() as block,
    ):
        out_sems = [out_s0, out_s1, out_s2, out_s3]
        PRO = 5  # DVE prologue ops before the first TT (4 u-chain + u_rep8)

        @block.gpsimd
        def _(gpsimd):
            gpsimd.dma_start(beta_t[:], beta[:]).then_inc(s_beta, 16)
            gpsimd.dma_start(
                xt[:, : xh * WCHUNK * D], x[:, : xh * WCHUNK * D]
            ).then_inc(s_x0, 16)
            if n_chunks > xh:
                gpsimd.dma_start(
                    xt[:, xh * WCHUNK * D :], x[:, xh * WCHUNK * D :]
                ).then_inc(s_x1, 16)
            # u bounce: SBUF [64,16] -> DRAM flat [1024]
            gpsimd.wait_ge(v_sem, 4)
            gpsimd.dma_start(u_dram[:], u_small[:]).then_inc(s_ub, 16)
            gpsimd.wait_ge(s_ub, 16)
            # partition-broadcast read back: [1024] -> [128, 1024]
            gpsimd.dma_start(
                u_rep[:], u_dram[:].unsqueeze(0).broadcast_to([H, D * C])
            ).then_inc(s_ur, 16)

        @block.vector
        def _(vector):
            vector.wait_ge(s_beta, 16)
            vector.tensor_mul(b2[:], beta_t[:], beta_t[:]).then_inc(v_sem)
            vector.wait_ge(v_sem, 1)  # DVE pipeline RAW drain
            vector.reduce_sum(ssum[:], b2[:], axis=mybir.AxisListType.X).then_inc(
                v_sem
            )
            vector.wait_ge(v_sem, 2)
            vector.reciprocal(rinv[:], ssum[:]).then_inc(v_sem)
            vector.wait_ge(v_sem, 3)
            vector.tensor_scalar(
                u_small[:], b2[:], rinv[:], None, op0=mybir.AluOpType.mult
            ).then_inc(v_sem)
            vector.wait_ge(s_ur, 16)
            # replicate u across the WCHUNK w's of a chunk: [128, 8*1024]
            vector.tensor_copy(
                u_rep8[:].rearrange("p (w e) -> p w e", w=WCHUNK),
                u_rep[:].unsqueeze(1).broadcast_to([H, WCHUNK, D * C]),
            ).then_inc(v_sem)
            vector.wait_ge(v_sem, PRO)  # u_rep8 drained before first TT reads it
            for r in range(repeat):
                for k in range(n_chunks):
                    if r == 0 and k == 0:
                        vector.wait_ge(s_x0, 16)
                    if r == 0 and k == xh and n_chunks > xh:
                        vector.wait_ge(s_x1, 16)
                    kk = r * n_chunks + k
                    if kk >= NBUF:
                        # slot reuse: wait until this slot's previous DMA
                        # (chunk kk-NBUF) fully drained it
                        vector.wait_ge(out_sems[kk % NBUF], 16 * (kk // NBUF))
                    slot = kk % NBUF
                    # iteration order (c outer, wi inner): the long contiguous
                    # inner dim runs at ~1 elem/cycle; a 16-long inner dim
                    # pays a large per-row restart penalty (3.6x slower).
                    in0 = (
                        xt[:, k * WCHUNK * D : (k + 1) * WCHUNK * D]
                        .unsqueeze(1)
                        .broadcast_to([H, C, WCHUNK * D])
                    )
                    in1 = u_rep8[:].rearrange("p (wi c) -> p c wi", c=C)
                    o2 = ot[:, slot * csz : (slot + 1) * csz].rearrange(
                        "p (wi c) -> p c wi", c=C
                    )
                    vector.tensor_mul(o2, in0, in1).then_inc(v_sem)

        def store_stream(eng, parity):
            cnt = [0, 0, 0, 0]
            for r in range(repeat):
                for k in range(parity, n_chunks, 2):
                    kk = r * n_chunks + k
                    eng.wait_ge(v_sem, PRO + kk + 1)
                    slot = kk % NBUF
                    eng.dma_start(
                        out[:, k * csz : (k + 1) * csz],
                        ot[:, slot * csz : (slot + 1) * csz],
                    ).then_inc(out_sems[slot], 16)
                    cnt[slot] += 1
            for s in range(NBUF):
                if cnt[s]:
                    eng.wait_ge(out_sems[s], 16 * cnt[s])

        @block.sync
        def _(sync):
            store_stream(sync, 0)

        @block.scalar
        def _(scalar):
            if n_chunks > 1:
                store_stream(scalar, 1)

    return nc


_NC_CACHE = {}


def get_bass(w_total=W, repeat=1):
    key = (w_total, repeat)
    if key not in _NC_CACHE:
        _NC_CACHE[key] = build_bass(w_total=w_total, repeat=repeat)
    return _NC_CACHE[key]


def kernel(inputs, beta):
    inputs = np.ascontiguousarray(inputs, dtype=np.float32)
    beta = np.ascontiguousarray(beta, dtype=np.float32)
    assert inputs.shape == (B, H, W, D), inputs.shape
    assert beta.shape == (D, C), beta.shape

    nc = get_bass()
    in_maps = [
        {"x": inputs[b].reshape(H, W * D), "beta": beta} for b in range(B)
    ]
    res = run_bass_kernel_spmd(nc, in_maps, core_ids=list(range(N_CORES)))
    return np.stack(
        [res.results[b]["out"].reshape(H, W, D, C) for b in range(B)], axis=0
    )
